# revision 36
# baseline (speedup 1.0000x reference)
"""AVWGCN (adaptive-adjacency graph conv) Trainium2 kernel, 8-core SPMD.

Math (reference):
    A = softmax(relu(E @ E.T), axis=1)            # [N, N], N=4096
    support_set = [I, A, 2*A@A - I]               # Chebyshev K=3
    x_g[b,n,k,c] = sum_m S[k,n,m] x[b,m,c]
    W[n,k,i,o]   = sum_d E[n,d] Wp[d,k,i,o]
    out[b,n,o]   = sum_{k,i} x_g[b,n,k,i] W[n,k,i,o] + (E @ bp)[n,o]

Key algebraic restructuring (avoids materializing A@A and S entirely):
    y0 = x;  y1 = A @ x;  y2 = 2*A@y1 - x
    softmax denominators folded in as row scales (U = exp(relu(z)) kept
    unnormalized in SBUF; exp(relu(z)) == max(1, exp(z))).
    Final per-node hypernet contraction re-associated through d:
    out[b,n,o] = sum_d E[n,d] * (sum_ki xg[b,n,ki] * Wp[d,ki,o]) + bias.

Sharding: node rows (n) split 512/core across 8 cores. U rows stay in
SBUF (2MB/core in bf16); one AllGather of normalized y1 (the only
cross-core traffic) via DRAM bounce buffers.
"""

import numpy as np

import concourse.bacc as bacc
import concourse.mybir as mybir
import concourse.tile as tile
import concourse.bass_utils as bass_utils

N_CORES = 8
B, N, C, O, D = 4, 4096, 16, 16, 10
KCH = 3                      # Chebyshev K
NC_ROWS = N // N_CORES       # 512 rows per core
MT = 128                     # partition tile
N_MT = N // MT               # 32 m-tiles over the full node dim
N_CH = NC_ROWS // MT         # 4 chunks of local rows
BC = B * C                   # 64
XE = BC + 1                  # 65 (ones column for row sums)

F32 = mybir.dt.float32
BF16 = mybir.dt.bfloat16

# --- configuration (dtype experiments) ---------------------------------
import os as _os
_VAR = _os.environ.get("AVW_VARIANT", "v3z")
CFG = {
    "v1": {"u_dt": F32, "logit_dt": F32, "y1_dt": F32, "logit_f32r": False},
    "v2": {"u_dt": BF16, "logit_dt": F32, "y1_dt": BF16, "logit_f32r": False},
    "v2r": {"u_dt": BF16, "logit_dt": mybir.dt.float32r, "y1_dt": BF16,
            "logit_f32r": False},
    # v3: transposed p1/p2 (U as moving operand), consolidated exp,
    # unnormalized y1+s allgather, DMA-assembled final contraction.
    "v3": {"u_dt": BF16, "logit_dt": mybir.dt.float32r, "y1_dt": BF16,
           "zmm_dt": F32},
    "v3z": {"u_dt": BF16, "logit_dt": mybir.dt.float32r, "y1_dt": BF16,
            "zmm_dt": BF16},
}[_VAR]

_CACHE: dict = {}


def _np_dt(dt):
    import ml_dtypes
    return np.dtype(ml_dtypes.bfloat16) if dt == BF16 else np.dtype(np.float32)


def build_program(cfg=CFG):
    u_dt, logit_dt, y1_dt = cfg["u_dt"], cfg["logit_dt"], cfg["y1_dt"]
    f32r = mybir.dt.float32r
    nc = bacc.Bacc("TRN2", target_bir_lowering=False, debug=False,
                   num_devices=N_CORES)

    # kernel I/O (per-core; host prepares layouts)
    et_d = nc.dram_tensor("et", [D, N], logit_dt, kind="ExternalInput").ap()
    ern_d = nc.dram_tensor("ern", [D, NC_ROWS], logit_dt, kind="ExternalInput").ap()
    ernb_d = nc.dram_tensor("ernb", [D, NC_ROWS], F32, kind="ExternalInput").ap()
    xe_d = nc.dram_tensor("xe", [MT, N_MT * XE], u_dt, kind="ExternalInput").ap()
    erow_d = nc.dram_tensor("erow", [MT, N_CH * D], F32, kind="ExternalInput").ap()
    xo_d = nc.dram_tensor("xo", [MT, N_CH * BC], F32, kind="ExternalInput").ap()
    wp2_d = nc.dram_tensor("wp2", [KCH * C, O * D], F32, kind="ExternalInput").ap()
    bp_d = nc.dram_tensor("bp", [D, O], F32, kind="ExternalInput").ap()
    id_d = nc.dram_tensor("ident", [MT, MT], F32, kind="ExternalInput").ap()
    out_d = nc.dram_tensor("out", [B, NC_ROWS, O], F32, kind="ExternalOutput").ap()

    with tile.TileContext(nc) as tc:
        with tc.tile_pool(name="const", bufs=1) as cp, \
             tc.tile_pool(name="work", bufs=3) as wp, \
             tc.tile_pool(name="psmm", bufs=2, space="PSUM") as psmm, \
             tc.tile_pool(name="pst", bufs=2, space="PSUM") as pst, \
             tc.tile_pool(name="psz", bufs=2, space="PSUM") as psz, \
             tc.tile_pool(name="dram", bufs=1, space="DRAM") as dp:

            # ---- persistent SBUF tensors ----
            et = cp.tile([D, N], logit_dt, name="et")
            ern = cp.tile([D, NC_ROWS], logit_dt, name="ern")
            ernb = cp.tile([D, NC_ROWS], F32, name="ernb")
            xe = cp.tile([MT, N_MT * XE], u_dt, name="xe")
            erow = cp.tile([MT, N_CH * D], F32, name="erow")
            wp2 = cp.tile([KCH * C, O * D], F32, name="wp2")
            bp = cp.tile([D, O], F32, name="bp")
            ident = cp.tile([MT, MT], F32, name="ident")
            # xg layout: [p, ch, b, k, c] so the per-(ch,b) transpose input
            # xg[:, ch, b, :, :] is one contiguous [128, 48] slice.
            xg = cp.tile([MT, N_CH * B * KCH * C], F32, name="xg")  # [128, 768]
            r2 = cp.tile([MT, N_CH], F32, name="r2")
            bias = cp.tile([MT, N_CH * O], F32, name="bias")
            us = [cp.tile([MT, NC_ROWS], u_dt, name=f"u{t}") for t in range(N_MT)]
            y1f = [cp.tile([MT, 8 * BC], y1_dt, name=f"y1f{g}") for g in range(4)]
            y1b = [cp.tile([MT, BC], y1_dt, name=f"y1b{ch}") for ch in range(N_CH)]

            bounce = dp.tile([NC_ROWS, BC], y1_dt, name="bounce")
            agout = dp.tile([N, BC], y1_dt, addr_space="Shared", name="agout")

            # ---- load inputs (E first: phase A starts earliest) ----
            nc.sync.dma_start(et[:], et_d)
            nc.sync.dma_start(ern[:], ern_d)
            nc.sync.dma_start(ernb[:], ernb_d)
            nc.sync.dma_start(wp2[:], wp2_d)
            nc.sync.dma_start(bp[:], bp_d)
            nc.sync.dma_start(ident[:], id_d)
            nc.sync.dma_start(erow[:], erow_d)
            nc.sync.dma_start(xe[:], xe_d)
            # x rows for this core -> xg slots k=0
            xg5 = xg[:].rearrange("p (ch b k c) -> p ch b k c",
                                  ch=N_CH, b=B, k=KCH)
            nc.sync.dma_start(
                xg5[:, :, :, 0, :],
                xo_d.rearrange("p (ch b c) -> p ch b c", ch=N_CH, b=B))

            # ---- bias = (E_rows @ bp) : [n, o] per chunk ----
            for ch in range(N_CH):
                pb = pst.tile([MT, O], F32, tag="t", name=f"pb{ch}")
                nc.tensor.matmul(pb[:], ernb[:, ch * MT:(ch + 1) * MT], bp[:],
                                 start=True, stop=True)
                nc.scalar.copy(bias[:, ch * O:(ch + 1) * O], pb[:])

            # ---- phase A: U_T[m, n_local] = max(1, exp(E[m].E[n])) ----
            for mt in range(N_MT):
                pa = psmm.tile([MT, NC_ROWS], F32, tag="mm", name=f"pa{mt}")
                lhsA = et[:, mt * MT:(mt + 1) * MT]
                rhsA = ern[:]
                if cfg.get("logit_f32r"):
                    lhsA = lhsA.bitcast(f32r)
                    rhsA = rhsA.bitcast(f32r)
                nc.tensor.matmul(pa[:], lhsA, rhsA, start=True, stop=True)
                nc.scalar.activation(us[mt][:], pa[:],
                                     mybir.ActivationFunctionType.Exp)
                nc.vector.tensor_scalar_max(us[mt][:], us[mt][:], 1.0)

            # ---- p1: y1[n] = (1/s) * sum_m U_T[m,n] x[m]; s via ones col ----
            for ch in range(N_CH):
                p1 = psmm.tile([MT, XE], F32, tag="mm", name=f"p1_{ch}")
                for mt in range(N_MT):
                    nc.tensor.matmul(p1[:],
                                     us[mt][:, ch * MT:(ch + 1) * MT],
                                     xe[:, mt * XE:(mt + 1) * XE],
                                     start=(mt == 0), stop=(mt == N_MT - 1))
                rec = wp.tile([MT, 1], F32, tag="rec")
                nc.vector.reciprocal(rec[:], p1[:, BC:BC + 1])
                slot1 = xg5[:, ch, :, 1, :]                       # [p, B, C]
                nc.vector.tensor_scalar_mul(
                    slot1, p1[:, 0:BC].rearrange("p (b c) -> p b c", b=B),
                    rec[:])
                nc.vector.tensor_scalar_mul(r2[:, ch:ch + 1], rec[:], 2.0)
                nc.vector.tensor_copy(
                    y1b[ch][:].rearrange("p (b c) -> p b c", b=B),
                    slot1)  # (cast) for AG
                nc.sync.dma_start(bounce[:][ch * MT:(ch + 1) * MT, :], y1b[ch][:])

            # ---- AllGather y1 across the 8 cores ----
            nc.gpsimd.collective_compute(
                "AllGather", mybir.AluOpType.bypass,
                replica_groups=[list(range(N_CORES))],
                ins=[bounce[:].opt()], outs=[agout[:].opt()])
            agv = agout[:].rearrange("(t p) c -> p t c", p=MT)  # [128, 32, 64]
            for g in range(4):
                nc.sync.dma_start(
                    y1f[g][:].rearrange("p (t c) -> p t c", t=8),
                    agv[:, g * 8:(g + 1) * 8, :])

            # ---- p2: y2[n] = 2/s * sum_m U_T[m,n] y1[m] - x[n] ----
            for ch in range(N_CH):
                p2 = psmm.tile([MT, BC], F32, tag="mm", name=f"p2_{ch}")
                for mt in range(N_MT):
                    nc.tensor.matmul(p2[:],
                                     us[mt][:, ch * MT:(ch + 1) * MT],
                                     y1f[mt // 8][:, (mt % 8) * BC:(mt % 8 + 1) * BC],
                                     start=(mt == 0), stop=(mt == N_MT - 1))
                nc.vector.scalar_tensor_tensor(
                    xg5[:, ch, :, 2, :],
                    p2[:].rearrange("p (b c) -> p b c", b=B),
                    r2[:, ch:ch + 1],
                    xg5[:, ch, :, 0, :],
                    op0=mybir.AluOpType.mult, op1=mybir.AluOpType.subtract)

            # ---- final: out[b,n,o] = sum_d E[n,d] Z[b,n,o,d] + bias ----
            for ch in range(N_CH):
                pz = psz.tile([MT, 1024], F32, tag="z", name=f"pz{ch}")
                for b in range(B):
                    ptr = pst.tile([KCH * C, MT], F32, tag="t")
                    base = (ch * B + b) * KCH * C
                    nc.tensor.transpose(ptr[:], xg[:, base:base + KCH * C],
                                        ident[:])
                    xgT = wp.tile([KCH * C, MT], F32, tag="xgT")
                    nc.scalar.copy(xgT[:], ptr[:])
                    nc.tensor.matmul(pz[:, b * 256:b * 256 + O * D],
                                     xgT[:], wp2[:], start=True, stop=True)
                zz = wp.tile([MT, B * O * D], F32, tag="zz")
                zzv = zz[:].rearrange("p (b o d) -> p b o d", b=B, o=O)
                pzv = pz[:].rearrange("p (b r) -> p b r", b=B)[:, :, 0:O * D] \
                    .rearrange("p b (o d) -> p b o d", o=O)
                ebc = erow[:, ch * D:(ch + 1) * D].unsqueeze(1).unsqueeze(1) \
                    .broadcast_to([MT, B, O, D])
                nc.vector.tensor_mul(zzv, pzv, ebc)
                osum = wp.tile([MT, B * O], F32, tag="osum")
                osv = osum[:].rearrange("p (b o) -> p b o", b=B)
                nc.vector.reduce_sum(osv, zzv, axis=mybir.AxisListType.X)
                ob2 = wp.tile([MT, B * O], F32, tag="ob2")
                obv = ob2[:].rearrange("p (b o) -> p b o", b=B)
                bv = bias[:, ch * O:(ch + 1) * O].unsqueeze(1) \
                    .broadcast_to([MT, B, O])
                nc.vector.tensor_add(obv, osv, bv)
                nc.sync.dma_start(
                    out_d[:, ch * MT:(ch + 1) * MT, :].transpose([1, 0, 2]),
                    obv)

    nc.compile()
    return nc


def build_v3(cfg=CFG):
    """Transposed p1/p2 pipeline: U is always the (wide) moving operand.

    The p1/p2 contractions and their epilogues are split into two
    column halves (n-ranges) so each half's normalize / y2 / assembly
    overlaps the other half's matmul chain.
    """
    u_dt, logit_dt, y1_dt = cfg["u_dt"], cfg["logit_dt"], cfg["y1_dt"]
    z_dt = cfg["zmm_dt"]
    HN = NC_ROWS // 2          # 256: half of the local n-range
    nc = bacc.Bacc("TRN2", target_bir_lowering=False, debug=False,
                   num_devices=N_CORES)

    et_d = nc.dram_tensor("et", [D, N], logit_dt, kind="ExternalInput").ap()
    ern_d = nc.dram_tensor("ern", [D, NC_ROWS], logit_dt, kind="ExternalInput").ap()
    xe_d = nc.dram_tensor("xe", [MT, N_MT * XE], u_dt, kind="ExternalInput").ap()
    xot_d = nc.dram_tensor("xot", [BC, NC_ROWS], z_dt, kind="ExternalInput").ap()
    erow_d = nc.dram_tensor("erow", [MT, N_CH * D], z_dt, kind="ExternalInput").ap()
    wp2_d = nc.dram_tensor("wp2", [KCH * C + 1, O * D], z_dt, kind="ExternalInput").ap()
    ones_d = nc.dram_tensor("ones64", [1, BC], y1_dt, kind="ExternalInput").ap()
    onesr_d = nc.dram_tensor("onesrow", [1, NC_ROWS], z_dt, kind="ExternalInput").ap()
    out_d = nc.dram_tensor("out", [N_CH, MT, B * O], F32,
                           kind="ExternalOutput").ap()

    NSUP = N_MT // 2  # 16 super-tiles of [128, 1024]

    with tile.TileContext(nc) as tc:
        with tc.tile_pool(name="const", bufs=1) as cp, \
             tc.tile_pool(name="work", bufs=3) as wp, \
             tc.tile_pool(name="dram", bufs=1, space="DRAM") as dp:

            # persistent SBUF
            et = cp.tile([D, N], logit_dt, name="et")
            ern = cp.tile([D, NC_ROWS], logit_dt, name="ern")
            xe = cp.tile([MT, N_MT * XE], u_dt, name="xe")
            xot = cp.tile([BC, NC_ROWS], z_dt, name="xot")
            erow = cp.tile([MT, N_CH * D], z_dt, name="erow")
            wp2 = cp.tile([KCH * C + 1, O * D], z_dt, name="wp2")
            ones64 = cp.tile([1, BC], y1_dt, name="ones64")
            us = [cp.tile([MT, 1024], u_dt, name=f"us{j}") for j in range(NSUP)]
            y1g = [cp.tile([MT, N_CORES * BC], y1_dt, name=f"y1g{t}")
                   for t in range(N_CH)]
            y1nt = [cp.tile([BC, HN], z_dt, name=f"y1nt{h}") for h in range(2)]
            y1tb = [cp.tile([BC, HN], y1_dt, name=f"y1tb{h}") for h in range(2)]
            srow = [cp.tile([1, HN], y1_dt, name=f"srow{h}") for h in range(2)]
            recbc = [cp.tile([BC, HN], F32, name=f"recbc{h}") for h in range(2)]
            y2t = [cp.tile([BC, HN], z_dt, name=f"y2t{h}") for h in range(2)]
            # per-(half, batch) stationary [49, 256] for the final matmul
            xgb = [[cp.tile([KCH * C + 1, HN], z_dt, name=f"xgb{h}_{b}")
                    for b in range(B)] for h in range(2)]

            bounce = dp.tile([BC, NC_ROWS], y1_dt, name="bounce")
            agout = dp.tile([N_CORES * BC, NC_ROWS], y1_dt,
                            addr_space="Shared", name="agout")

            # ---- input loads ----
            nc.sync.dma_start(ern[:], ern_d)
            for q in range(4):
                nc.sync.dma_start(et[:, q * (N // 4):(q + 1) * (N // 4)],
                                  et_d[:, q * (N // 4):(q + 1) * (N // 4)])
            nc.sync.dma_start(xe[:], xe_d)
            nc.sync.dma_start(wp2[:], wp2_d)
            nc.sync.dma_start(ones64[:], ones_d)
            nc.sync.dma_start(erow[:], erow_d)
            nc.sync.dma_start(xot[:], xot_d)
            # k=0 rows straight from x; row 48 = ones (bias row of wp2)
            for h in range(2):
                for b in range(B):
                    nc.sync.dma_start(
                        xgb[h][b][:][0:C, :],
                        xot_d[b * C:(b + 1) * C, h * HN:(h + 1) * HN])
                    nc.sync.dma_start(
                        xgb[h][b][:][KCH * C:KCH * C + 1, :],
                        onesr_d[:, h * HN:(h + 1) * HN])

            with tc.tile_pool(name="psA", bufs=2, space="PSUM") as psA:
                # ---- phase A: U_T[m, n_local] = max(1, exp(E.E^T)) ----
                for j in range(NSUP):
                    pa = psA.tile([MT, 1024], F32, tag="mm", name=f"pa{j}")
                    for q in range(2):
                        mt = 2 * j + q
                        nc.tensor.matmul(pa[:, q * 512:(q + 1) * 512],
                                         et[:, mt * MT:(mt + 1) * MT], ern[:],
                                         start=True, stop=True)
                    nc.scalar.activation(us[j][:], pa[:],
                                         mybir.ActivationFunctionType.Exp)
                    nc.vector.tensor_scalar_max(us[j][:], us[j][:], 1.0)

                # ---- p1T: [x|1].T @ U per half; normalize overlaps ----
                psum1 = [psA.tile([XE, HN], F32, tag=f"p1_{h}", bufs=1,
                                  name=f"psum1{h}") for h in range(2)]
                for h in range(2):
                    for mt in range(N_MT):
                        nc.tensor.matmul(
                            psum1[h][:], xe[:, mt * XE:(mt + 1) * XE],
                            us[mt // 2][:, (mt % 2) * 512 + h * HN:
                                        (mt % 2) * 512 + (h + 1) * HN],
                            start=(mt == 0), stop=(mt == N_MT - 1))
                    # normalize: 1/s broadcast to all 64 rows via rank-1 matmul
                    with nc.allow_low_precision(reason="bf16 softmax scale"):
                        nc.vector.reciprocal(srow[h][:], psum1[h][:][BC:XE, :])
                    sbc = psA.tile([BC, HN], F32, tag="sbc", bufs=2,
                                   name=f"sbc{h}")
                    nc.tensor.matmul(sbc[:], ones64[:], srow[h][:],
                                     start=True, stop=True)
                    nc.scalar.copy(recbc[h][:], sbc[:])
                    nc.vector.tensor_mul(y1nt[h][:], psum1[h][:][0:BC, :],
                                         recbc[h][:])
                    if z_dt == y1_dt:
                        nc.sync.dma_start(bounce[:][:, h * HN:(h + 1) * HN],
                                          y1nt[h][:])
                    else:
                        nc.scalar.copy(y1tb[h][:], y1nt[h][:])
                        nc.sync.dma_start(bounce[:][:, h * HN:(h + 1) * HN],
                                          y1tb[h][:])
                for h in range(2):
                    for b in range(B):
                        nc.sync.dma_start(xgb[h][b][:][C:2 * C, :],
                                          y1nt[h][:][b * C:(b + 1) * C, :])

            # ---- AllGather normalized y1 (transposed layout) ----
            nc.gpsimd.collective_compute(
                "AllGather", mybir.AluOpType.bypass,
                replica_groups=[list(range(N_CORES))],
                ins=[bounce[:].opt()], outs=[agout[:].opt()])

            with tc.tile_pool(name="psB", bufs=1, space="PSUM") as psB, \
                 tc.tile_pool(name="psz", bufs=2, space="PSUM") as psz:
                # transpose-readback: y1g[t][p, r*64+bc] = agout[r*64+bc, t*128+p]
                for t in range(N_CH):
                    nc.sync.dma_start_transpose(
                        y1g[t][:], agout[:][:, t * MT:(t + 1) * MT])

                psum2 = [psB.tile([BC, HN], F32, name=f"psum2{h}")
                         for h in range(2)]
                # both p2T chains first: PE stays dense (h1 chain runs while
                # h0's DVE/DMA tail is in flight)
                for h in range(2):
                    for i, mt in enumerate(
                            r * N_CH + tl for tl in range(N_CH)
                            for r in range(N_CORES)):
                        nc.tensor.matmul(
                            psum2[h][:],
                            y1g[mt % N_CH][:, (mt // N_CH) * BC:
                                           (mt // N_CH + 1) * BC],
                            us[mt // 2][:, (mt % 2) * 512 + h * HN:
                                        (mt % 2) * 512 + (h + 1) * HN],
                            start=(i == 0), stop=(i == N_MT - 1))
                for h in range(2):
                    # y2 = 2*(1/s)*y2pre - x
                    y2tmp = wp.tile([BC, HN], F32, tag="y2tmp")
                    nc.vector.tensor_mul(y2tmp[:], psum2[h][:], recbc[h][:])
                    nc.vector.scalar_tensor_tensor(
                        y2t[h][:], y2tmp[:], 2.0,
                        xot[:][:, h * HN:(h + 1) * HN],
                        op0=mybir.AluOpType.mult, op1=mybir.AluOpType.subtract)
                    for b in range(B):
                        nc.sync.dma_start(xgb[h][b][:][2 * C:3 * C, :],
                                          y2t[h][:][b * C:(b + 1) * C, :])

                # ---- final contraction + E-combine ----
                for ch in range(N_CH):
                    h, cc = ch // 2, ch % 2
                    pz = psz.tile([MT, 1024], F32, tag="z", name=f"pz{ch}")
                    for b in range(B):
                        nc.tensor.matmul(
                            pz[:, b * 256:b * 256 + O * D],
                            xgb[h][b][:, cc * MT:(cc + 1) * MT],
                            wp2[:], start=True, stop=True)
                    zsb = wp.tile([MT, B * O * D], z_dt, tag="zsb")
                    nc.scalar.copy(zsb[:], pz[:].rearrange(
                        "p (b r) -> p b r", b=B)[:, :, 0:O * D])
                    zz = wp.tile([MT, B * O * D], z_dt, tag="zz")
                    zzv = zz[:].rearrange("p (b o d) -> p b o d", b=B, o=O)
                    ebc = erow[:, ch * D:(ch + 1) * D].unsqueeze(1) \
                        .unsqueeze(1).broadcast_to([MT, B, O, D])
                    nc.vector.tensor_mul(
                        zzv,
                        zsb[:].rearrange("p (b o d) -> p b o d", b=B, o=O),
                        ebc)
                    osum = wp.tile([MT, B * O], F32, tag="osum")
                    osv = osum[:].rearrange("p (b o) -> p b o", b=B)
                    nc.vector.reduce_sum(osv, zzv, axis=mybir.AxisListType.X)
                    nc.scalar.dma_start(out_d[ch], osum[:])

    nc.compile()
    return nc


def build_v3(cfg=CFG):
    """Transposed p1/p2 pipeline: U is always the (wide) moving operand.

    The p1/p2 contractions and their epilogues are split into two
    column halves (n-ranges) so each half's normalize / y2 / assembly
    overlaps the other half's matmul chain.
    """
    u_dt, logit_dt, y1_dt = cfg["u_dt"], cfg["logit_dt"], cfg["y1_dt"]
    z_dt = cfg["zmm_dt"]
    HN = NC_ROWS // 2          # 256: half of the local n-range
    nc = bacc.Bacc("TRN2", target_bir_lowering=False, debug=False,
                   num_devices=N_CORES)

    et_d = nc.dram_tensor("et", [D, N], logit_dt, kind="ExternalInput").ap()
    ern_d = nc.dram_tensor("ern", [D, NC_ROWS], logit_dt, kind="ExternalInput").ap()
    xe_d = nc.dram_tensor("xe", [MT, N_MT * XE], u_dt, kind="ExternalInput").ap()
    xot_d = nc.dram_tensor("xot", [BC, NC_ROWS], z_dt, kind="ExternalInput").ap()
    erow_d = nc.dram_tensor("erow", [MT, N_CH * D], z_dt, kind="ExternalInput").ap()
    wp2_d = nc.dram_tensor("wp2", [KCH * C + 1, O * D], z_dt, kind="ExternalInput").ap()
    ones_d = nc.dram_tensor("ones64", [1, BC], y1_dt, kind="ExternalInput").ap()
    onesr_d = nc.dram_tensor("onesrow", [1, NC_ROWS], z_dt, kind="ExternalInput").ap()
    out_d = nc.dram_tensor("out", [N_CH, MT, B * O], F32,
                           kind="ExternalOutput").ap()

    NSUP = N_MT // 2  # 16 super-tiles of [128, 1024]

    with tile.TileContext(nc) as tc:
        with tc.tile_pool(name="const", bufs=1) as cp, \
             tc.tile_pool(name="work", bufs=3) as wp, \
             tc.tile_pool(name="dram", bufs=1, space="DRAM") as dp:

            # persistent SBUF
            et = cp.tile([D, N], logit_dt, name="et")
            ern = cp.tile([D, NC_ROWS], logit_dt, name="ern")
            xe = cp.tile([MT, N_MT * XE], u_dt, name="xe")
            xot = cp.tile([BC, NC_ROWS], z_dt, name="xot")
            erow = cp.tile([MT, N_CH * D], z_dt, name="erow")
            wp2 = cp.tile([KCH * C + 1, O * D], z_dt, name="wp2")
            ones64 = cp.tile([1, BC], y1_dt, name="ones64")
            us = [cp.tile([MT, 1024], u_dt, name=f"us{j}") for j in range(NSUP)]
            y1g = [cp.tile([MT, N_CORES * BC], y1_dt, name=f"y1g{t}")
                   for t in range(N_CH)]
            y1nt = [cp.tile([BC, HN], z_dt, name=f"y1nt{h}") for h in range(2)]
            y1tb = [cp.tile([BC, HN], y1_dt, name=f"y1tb{h}") for h in range(2)]
            srow = [cp.tile([1, HN], y1_dt, name=f"srow{h}") for h in range(2)]
            recbc = [cp.tile([BC, HN], F32, name=f"recbc{h}") for h in range(2)]
            y2t = [cp.tile([BC, HN], z_dt, name=f"y2t{h}") for h in range(2)]
            # per-(half, batch) stationary [49, 256] for the final matmul
            xgb = [[cp.tile([KCH * C + 1, HN], z_dt, name=f"xgb{h}_{b}")
                    for b in range(B)] for h in range(2)]

            bounce = dp.tile([BC, NC_ROWS], y1_dt, name="bounce")
            agout = dp.tile([N_CORES * BC, NC_ROWS], y1_dt,
                            addr_space="Shared", name="agout")

            # ---- input loads ----
            nc.sync.dma_start(ern[:], ern_d)
            for q in range(4):
                nc.sync.dma_start(et[:, q * (N // 4):(q + 1) * (N // 4)],
                                  et_d[:, q * (N // 4):(q + 1) * (N // 4)])
            nc.sync.dma_start(xe[:], xe_d)
            nc.sync.dma_start(wp2[:], wp2_d)
            nc.sync.dma_start(ones64[:], ones_d)
            nc.sync.dma_start(erow[:], erow_d)
            nc.sync.dma_start(xot[:], xot_d)
            # k=0 rows straight from x; row 48 = ones (bias row of wp2)
            for h in range(2):
                for b in range(B):
                    nc.sync.dma_start(
                        xgb[h][b][:][0:C, :],
                        xot_d[b * C:(b + 1) * C, h * HN:(h + 1) * HN])
                    nc.sync.dma_start(
                        xgb[h][b][:][KCH * C:KCH * C + 1, :],
                        onesr_d[:, h * HN:(h + 1) * HN])

            with tc.tile_pool(name="psA", bufs=2, space="PSUM") as psA:
                # ---- phase A: U_T[m, n_local] = max(1, exp(E.E^T)) ----
                for j in range(NSUP):
                    pa = psA.tile([MT, 1024], F32, tag="mm", name=f"pa{j}")
                    for q in range(2):
                        mt = 2 * j + q
                        nc.tensor.matmul(pa[:, q * 512:(q + 1) * 512],
                                         et[:, mt * MT:(mt + 1) * MT], ern[:],
                                         start=True, stop=True)
                    nc.scalar.activation(us[j][:], pa[:],
                                         mybir.ActivationFunctionType.Exp)
                    nc.vector.tensor_scalar_max(us[j][:], us[j][:], 1.0)

                # ---- p1T: [x|1].T @ U per half; normalize overlaps ----
                psum1 = [psA.tile([XE, HN], F32, tag=f"p1_{h}", bufs=1,
                                  name=f"psum1{h}") for h in range(2)]
                for h in range(2):
                    for mt in range(N_MT):
                        nc.tensor.matmul(
                            psum1[h][:], xe[:, mt * XE:(mt + 1) * XE],
                            us[mt // 2][:, (mt % 2) * 512 + h * HN:
                                        (mt % 2) * 512 + (h + 1) * HN],
                            start=(mt == 0), stop=(mt == N_MT - 1))
                    # normalize: 1/s broadcast to all 64 rows via rank-1 matmul
                    with nc.allow_low_precision(reason="bf16 softmax scale"):
                        nc.vector.reciprocal(srow[h][:], psum1[h][:][BC:XE, :])
                    sbc = psA.tile([BC, HN], F32, tag="sbc", bufs=2,
                                   name=f"sbc{h}")
                    nc.tensor.matmul(sbc[:], ones64[:], srow[h][:],
                                     start=True, stop=True)
                    nc.scalar.copy(recbc[h][:], sbc[:])
                    nc.vector.tensor_mul(y1nt[h][:], psum1[h][:][0:BC, :],
                                         recbc[h][:])
                    if z_dt == y1_dt:
                        nc.sync.dma_start(bounce[:][:, h * HN:(h + 1) * HN],
                                          y1nt[h][:])
                    else:
                        nc.scalar.copy(y1tb[h][:], y1nt[h][:])
                        nc.sync.dma_start(bounce[:][:, h * HN:(h + 1) * HN],
                                          y1tb[h][:])
                for h in range(2):
                    for b in range(B):
                        nc.sync.dma_start(xgb[h][b][:][C:2 * C, :],
                                          y1nt[h][:][b * C:(b + 1) * C, :])

            # ---- AllGather normalized y1 (transposed layout) ----
            nc.gpsimd.collective_compute(
                "AllGather", mybir.AluOpType.bypass,
                replica_groups=[list(range(N_CORES))],
                ins=[bounce[:].opt()], outs=[agout[:].opt()])

            with tc.tile_pool(name="psB", bufs=1, space="PSUM") as psB, \
                 tc.tile_pool(name="psz", bufs=2, space="PSUM") as psz:
                # transpose-readback: y1g[t][p, r*64+bc] = agout[r*64+bc, t*128+p]
                for t in range(N_CH):
                    nc.sync.dma_start_transpose(
                        y1g[t][:], agout[:][:, t * MT:(t + 1) * MT])

                psum2 = [psB.tile([BC, HN], F32, name=f"psum2{h}")
                         for h in range(2)]
                for h in range(2):
                    # ---- p2T half: y1n.T @ U -> [64, 256] ----
                    for i, mt in enumerate(
                            r * N_CH + tl for tl in range(N_CH)
                            for r in range(N_CORES)):
                        nc.tensor.matmul(
                            psum2[h][:],
                            y1g[mt % N_CH][:, (mt // N_CH) * BC:
                                           (mt // N_CH + 1) * BC],
                            us[mt // 2][:, (mt % 2) * 512 + h * HN:
                                        (mt % 2) * 512 + (h + 1) * HN],
                            start=(i == 0), stop=(i == N_MT - 1))
                    # y2 = 2*(1/s)*y2pre - x
                    y2tmp = wp.tile([BC, HN], F32, tag="y2tmp")
                    nc.vector.tensor_mul(y2tmp[:], psum2[h][:], recbc[h][:])
                    nc.vector.scalar_tensor_tensor(
                        y2t[h][:], y2tmp[:], 2.0,
                        xot[:][:, h * HN:(h + 1) * HN],
                        op0=mybir.AluOpType.mult, op1=mybir.AluOpType.subtract)
                    for b in range(B):
                        nc.sync.dma_start(xgb[h][b][:][2 * C:3 * C, :],
                                          y2t[h][:][b * C:(b + 1) * C, :])

                    # ---- final contraction + E-combine (2 chunks/half) ----
                    for cc in range(2):
                        ch = h * 2 + cc
                        pz = psz.tile([MT, 1024], F32, tag="z", name=f"pz{ch}")
                        for b in range(B):
                            nc.tensor.matmul(
                                pz[:, b * 256:b * 256 + O * D],
                                xgb[h][b][:, cc * MT:(cc + 1) * MT],
                                wp2[:], start=True, stop=True)
                        zsb = wp.tile([MT, B * O * D], z_dt, tag="zsb")
                        nc.scalar.copy(zsb[:], pz[:].rearrange(
                            "p (b r) -> p b r", b=B)[:, :, 0:O * D])
                        zz = wp.tile([MT, B * O * D], z_dt, tag="zz")
                        zzv = zz[:].rearrange("p (b o d) -> p b o d", b=B, o=O)
                        ebc = erow[:, ch * D:(ch + 1) * D].unsqueeze(1) \
                            .unsqueeze(1).broadcast_to([MT, B, O, D])
                        nc.vector.tensor_mul(
                            zzv,
                            zsb[:].rearrange("p (b o d) -> p b o d", b=B, o=O),
                            ebc)
                        osum = wp.tile([MT, B * O], F32, tag="osum")
                        osv = osum[:].rearrange("p (b o) -> p b o", b=B)
                        nc.vector.reduce_sum(osv, zzv, axis=mybir.AxisListType.X)
                        nc.scalar.dma_start(out_d[ch], osum[:])

    nc.compile()
    return nc


def _prep_inputs_v3(x, node_embeddings, weights_pool, bias_pool, cfg=CFG):
    u_np = _np_dt(cfg["u_dt"])
    lg_np = _np_dt(cfg["logit_dt"])
    y1_np = _np_dt(cfg["y1_dt"])
    z_np = _np_dt(cfg["zmm_dt"])
    x = np.asarray(x, np.float32)
    E = np.asarray(node_embeddings, np.float32)
    Wp = np.asarray(weights_pool, np.float32)
    bp = np.asarray(bias_pool, np.float32)

    x_mat = np.ascontiguousarray(x.transpose(1, 0, 2).reshape(N, BC))
    xe = np.concatenate([x_mat, np.ones((N, 1), np.float32)], axis=1)
    xe_h = np.ascontiguousarray(
        xe.reshape(N_MT, MT, XE).transpose(1, 0, 2).reshape(MT, N_MT * XE)
    ).astype(u_np)
    et_h = np.ascontiguousarray(E.T).astype(lg_np)
    wp2_h = np.ascontiguousarray(np.concatenate([
        Wp.transpose(1, 2, 3, 0).reshape(KCH * C, O * D),
        bp.T.reshape(1, O * D),
    ], axis=0)).astype(z_np)
    ones_h = np.ones((1, BC), y1_np)

    in_maps = []
    for c in range(N_CORES):
        rows = slice(c * NC_ROWS, (c + 1) * NC_ROWS)
        Ec = E[rows]
        ern_h = np.ascontiguousarray(Ec.T)
        erow_h = np.ascontiguousarray(
            Ec.reshape(N_CH, MT, D).transpose(1, 0, 2).reshape(MT, N_CH * D))
        xot_h = np.ascontiguousarray(x_mat[rows].T).astype(z_np)
        in_maps.append({
            "et": et_h,
            "ern": ern_h.astype(lg_np),
            "xe": xe_h,
            "xot": xot_h,
            "erow": erow_h.astype(z_np),
            "wp2": wp2_h,
            "ones64": ones_h,
            "onesrow": np.ones((1, NC_ROWS), z_np),
        })
    return in_maps


def _prep_inputs(x, node_embeddings, weights_pool, bias_pool, cfg=CFG):
    """Host-side layout prep. Returns per-core input maps."""
    u_np = _np_dt(cfg["u_dt"])
    lg_np = _np_dt(cfg["logit_dt"])
    x = np.asarray(x, np.float32)
    E = np.asarray(node_embeddings, np.float32)
    Wp = np.asarray(weights_pool, np.float32)
    bp = np.asarray(bias_pool, np.float32)

    x_mat = np.ascontiguousarray(x.transpose(1, 0, 2).reshape(N, BC))
    xe = np.concatenate([x_mat, np.ones((N, 1), np.float32)], axis=1)  # [N,65]
    xe_h = np.ascontiguousarray(
        xe.reshape(N_MT, MT, XE).transpose(1, 0, 2).reshape(MT, N_MT * XE)
    ).astype(u_np)
    et_h = np.ascontiguousarray(E.T).astype(lg_np)
    wp2_h = np.ascontiguousarray(
        Wp.transpose(1, 2, 3, 0).reshape(KCH * C, O * D))
    id_h = np.eye(MT, dtype=np.float32)

    in_maps = []
    for c in range(N_CORES):
        rows = slice(c * NC_ROWS, (c + 1) * NC_ROWS)
        Ec = E[rows]                                   # [512, 10]
        ern_h = np.ascontiguousarray(Ec.T)
        erow_h = np.ascontiguousarray(
            Ec.reshape(N_CH, MT, D).transpose(1, 0, 2).reshape(MT, N_CH * D))
        xo_h = np.ascontiguousarray(
            x_mat[rows].reshape(N_CH, MT, BC).transpose(1, 0, 2)
            .reshape(MT, N_CH * BC))
        in_maps.append({
            "et": et_h,
            "ern": ern_h.astype(lg_np),
            "ernb": ern_h,
            "xe": xe_h,
            "erow": erow_h,
            "xo": xo_h,
            "wp2": wp2_h,
            "bp": bp,
            "ident": id_h,
        })
    return in_maps


_IS_V3 = _VAR.startswith("v3")


def kernel(x, node_embeddings, adjs, weights_pool, bias_pool):
    if "nc" not in _CACHE:
        _CACHE["nc"] = build_v3() if _IS_V3 else build_program()
    nc = _CACHE["nc"]
    prep = _prep_inputs_v3 if _IS_V3 else _prep_inputs
    in_maps = prep(x, node_embeddings, weights_pool, bias_pool)
    res = None
    last_exc = None
    for attempt in range(3):
        try:
            res = bass_utils.run_bass_kernel_spmd(
                nc, in_maps, core_ids=list(range(N_CORES)))
            break
        except Exception as e:  # transient NRT/device hiccups: retry
            last_exc = e
            import time
            time.sleep(2.0)
    if res is None:
        raise last_exc
    if _IS_V3:
        # per-core staging [N_CH, MT, B*O] -> [B, 512, O], concat on n
        parts = []
        for c in range(N_CORES):
            oc = res.results[c]["out"].reshape(N_CH, MT, B, O)
            parts.append(oc.transpose(2, 0, 1, 3).reshape(B, NC_ROWS, O))
        out = np.concatenate(parts, axis=1)
    else:
        out = np.concatenate([res.results[c]["out"] for c in range(N_CORES)],
                             axis=1)
    return np.ascontiguousarray(out).astype(np.float32)
